# revision 2
# baseline (speedup 1.0000x reference)
"""Trainium2 Bass kernel for nn_Attention_76450417868987 — v2.

Three Bahdanau-style additive attentions + gated fusion, pure data-parallel
over batch (512 -> 64 per core x 8 cores).

v2 design (vs v1 baseline at ~156us sim):
  - n-major flat row order r = n*64 + b everywhere: batch = r % 64, so the
    einsum indicator I2 = [I64; I64] and the hp-broadcast pattern G are
    small per-kernel constants (no 4MB indicator DMAs, no DRAM softmax
    bounce -- per-batch sums are a free-dim reduce + one tiny fold matmul).
  - Scores computed in TRANSPOSED layout: host supplies pT [d, (n b)] fp8.
    X.T = pT + hp.T-broadcast is ONE fp8 DoubleRow matmul per (chunk,
    group): ktile0 = identf8 @ pT (pass-through), ktile1 = hp8 @ G
    (broadcast), K=256 fused, 0.5 cyc/col.
  - tanh on ScalarE over whole 3-tile PSUM groups [128, 1536] (the
    designed bottleneck engine, ~73us).
  - Score dot moves to PE: ttT chunk as stationary, aw chunk as 1-column
    moving operand -> scol[:, t] directly in [row-partition, tile] layout.
    DVE is freed (v1 spent ~90us on scalar_tensor_tensor dots at 1x).
  - Einsum res += lt.T @ f as fp8 DoubleRow over tile PAIRS (lt2 [128,2,64],
    f2 [128,2,512]), 0.5 cyc/col.
  - PSUM: 2x3-bank X.T groups + 1 scol bank + 1 res/gate bank = 8.
"""

import os
import sys

if "/opt/trn_rl_repo" not in sys.path:
    sys.path.insert(0, "/opt/trn_rl_repo")

import numpy as np

B = 512
NA, NCP, NSW = 196, 50, 50
D = 512
M = 8
BL = B // M  # 64
P = 128
NT_A = BL * NA // P  # 98
NT_C = BL * NCP // P  # 25

SG_TILES = 9  # super-group: DMA granule for pT (1152 rows)
GRP_TILES = 3  # X.T/tanh group (384 rows, 3 PSUM banks)
EB_TILES = 12  # exp/lt/einsum batch

_CACHE = {}


def _tiles_split(nt, step):
    out = []
    t = 0
    while t < nt:
        out.append((t, min(step, nt - t)))
        t += step
    return out


def _build(nc, reps=1, mode="full"):
    import concourse.bass as bass  # noqa: F401
    from concourse import mybir
    from concourse.tile import TileContext

    f32 = mybir.dt.float32
    bf16 = mybir.dt.bfloat16
    f8 = mybir.dt.float8e4
    AF = mybir.ActivationFunctionType
    OP = mybir.AluOpType
    AX = mybir.AxisListType
    DR = mybir.MatmulPerfMode.DoubleRow

    def dp(name, shape, dt):
        return nc.declare_dram_parameter(name, shape, dt, isOutput=False)

    # big streams (fp8, host-prepped layouts)
    pT = {
        "c": dp("pT_cpt", [P, 4, NT_C * P], f8),
        "s": dp("pT_sw", [P, 4, NT_C * P], f8),
        "a": dp("pT_att", [P, 4, NT_A * P], f8),
    }
    fR = {
        "c": dp("f_cpt", [NT_C * P, D], f8),
        "s": dp("f_sw", [NT_C * P, D], f8),
        "a": dp("f_att", [NT_A * P, D], f8),
    }
    # projection weights (bf16) + biases (bf16 rows)
    wproj = {
        "c": dp("c_h2cpt_w", [D, D], bf16),
        "s": dp("s_h2word_w", [D, D], bf16),
        "a": dp("c_h2att_w", [D, D], bf16),
    }
    bproj = {
        "c": dp("c_h2cpt_b", [1, D], bf16),
        "s": dp("s_h2word_b", [1, D], bf16),
        "a": dp("c_h2att_b", [1, D], bf16),
    }
    awT_d = {
        "c": dp("awT_cpt", [P, 2, 2], f8),
        "s": dp("awT_sw", [P, 2, 2], f8),
        "a": dp("awT_att", [P, 2, 2], f8),
    }
    w_tc = dp("t_cont_w", [2 * D, D], bf16)
    w_ts = dp("t_senti_w", [2 * D, D], bf16)
    w_th = dp("t_h2att_w", [D, D], bf16)
    tb3_d = dp("tb3", [3, D], bf16)  # t_cont_b, t_senti_b, t_h2att_b
    alphaw_d = dp("t_alpha_w", [1, D], bf16)
    ab_d = dp("t_alpha_b", [1, 1], bf16)

    h_d = dp("h", [BL, D], f32)
    senti_d = dp("senti_feats", [BL, D], f32)

    identf_d = dp("identf32", [P, P], f32)
    A8base_d = dp("A8base", [P, 2, 4, P], f8)
    I2f8_d = dp("I2f8", [P, BL], f8)
    I2bf_d = dp("I2bf", [P, BL], bf16)
    G4_d = dp("G4", [P, 4, SG_TILES * P], f8)
    ones3_d = dp("ones3", [3, BL], bf16)

    out_d = nc.declare_dram_parameter("out", [BL, 2 * D], f32, isOutput=True)

    BRANCHES = [
        ("c", NT_C, 0),
        ("s", NT_C, NT_C),
        ("a", NT_A, 2 * NT_C),
    ]

    with TileContext(nc) as tc:
        with (
            tc.tile_pool(name="const", bufs=1) as constp,
            tc.tile_pool(name="ct", bufs=1) as ctp,
            tc.tile_pool(name="fio", bufs=3) as fiop,
            tc.tile_pool(name="tt", bufs=3) as ttp,
            tc.tile_pool(name="lt", bufs=6) as ltp,
            tc.tile_pool(name="wio", bufs=2) as wiop,
            tc.tile_pool(name="small", bufs=2) as smallp,
            tc.tile_pool(name="px", bufs=2, space="PSUM") as pxp,
            tc.tile_pool(name="pscol", bufs=1, space="PSUM") as pscolp,
            tc.tile_pool(name="psres", bufs=1, space="PSUM") as presp,
        ):
            for _rep in range(reps):
                # ---------------- consts ----------------
                identf = constp.tile([P, P], f32, tag="identf")
                nc.scalar.dma_start(identf[:], identf_d[:])
                I2f8 = constp.tile([P, BL], f8, tag="I2f8")
                nc.scalar.dma_start(I2f8[:], I2f8_d[:])
                I2bf = constp.tile([P, BL], bf16, tag="I2bf")
                nc.scalar.dma_start(I2bf[:], I2bf_d[:])
                ones3 = constp.tile([3, BL], bf16, tag="ones3")
                nc.scalar.dma_start(ones3[:], ones3_d[:])
                h_sb = constp.tile([BL, D], f32, tag="h_sb")
                nc.scalar.dma_start(h_sb[:], h_d[:])
                awT = {}
                for k in ("c", "s", "a"):
                    awT[k] = constp.tile([P, 2, 2], f8, tag=f"awT_{k}", name=f"awT_{k}")
                    nc.scalar.dma_start(awT[k][:], awT_d[k][:])
                ab_sb = constp.tile([1, 1], bf16, tag="ab_sb")
                nc.scalar.dma_start(ab_sb[:], ab_d[:])
                alphaw = constp.tile([1, D], bf16, tag="alphaw")
                nc.scalar.dma_start(alphaw[:], alphaw_d[:])

                # scol bank: dot columns 0..147, fold cols 160-166,
                # transpose scratch 256-319, alphab 0-511 (setup only),
                # ab_col 330
                scol = pscolp.tile([P, D], f32, tag="scol")

                # --- setup broadcasts via scol bank (before any dots) ---
                nc.tensor.matmul(
                    scol[:BL, :], ones3[:1, :], alphaw[:], start=True, stop=True
                )
                alphab = constp.tile([BL, D], bf16, tag="alphab")
                nc.vector.tensor_copy(alphab[:], scol[:BL, :])
                nc.tensor.matmul(
                    scol[:BL, 330:331], ones3[:1, :], ab_sb[:], start=True, stop=True
                )
                ab_col = constp.tile([BL, 1], f32, tag="ab_col")
                nc.vector.tensor_copy(ab_col[:], scol[:BL, 330:331])

                # hT chunks (for hp projections + gate)
                hT = constp.tile([P, 4, BL], bf16, tag="hT")
                for c in range(4):
                    nc.tensor.transpose(
                        scol[:, 256:320], h_sb[:, c * P : (c + 1) * P], identf[:BL, :BL]
                    )
                    nc.vector.tensor_copy(hT[:, c, :], scol[:, 256:320])

                # ---------------- A8 per branch ----------------
                # A8[k][:, kt, c, :]: kt=0 -> identf8, kt=1 -> hp8 chunk c
                # (hp8 = fp8 of h @ W + b, rows 64-127 zero)
                A8 = {}
                for k in ("c", "s", "a"):
                    A8[k] = constp.tile([P, 2, 4, P], f8, tag=f"A8_{k}", name=f"A8_{k}")
                    nc.sync.dma_start(A8[k][:], A8base_d[:])

                def hp_fill(k):
                    wt = wiop.tile([P, 4, D], bf16, tag="wproj")
                    nc.scalar.dma_start(
                        wt[:], wproj[k].rearrange("(c p) d -> p c d", p=P)
                    )
                    brow = smallp.tile([1, D], bf16, tag="brow")
                    nc.scalar.dma_start(brow[:], bproj[k][:1, :])
                    hp_ps = presp.tile([P, D], f32, tag="res")
                    for c in range(4):
                        nc.tensor.matmul(
                            hp_ps[:BL, :], hT[:, c, :], wt[:, c, :],
                            start=(c == 0), stop=False,
                        )
                    nc.tensor.matmul(
                        hp_ps[:BL, :], ones3[:1, :], brow[:], start=False, stop=True
                    )
                    # fp8 quantize into A8 ktile1
                    for c in range(4):
                        nc.vector.tensor_copy(
                            A8[k][:BL, 1, c, :], hp_ps[:BL, c * P : (c + 1) * P]
                        )

                for k in ("c", "s", "a"):
                    hp_fill(k)

                # ---------------- ct tiles (pT + G interleaved) ----------
                # ctb[buf] = [128, 4(chunk), 2(ktile), 1152] fp8;
                # [:,:,0,:] = pT stream (one DMA/super-group),
                # [:,:,1,:] = G broadcast pattern (filled once)
                NCTB = 2
                ctb = [
                    ctp.tile([P, 4, 2, SG_TILES * P], f8, tag=f"ctb{b}", name=f"ctb{b}")
                    for b in range(NCTB)
                ]
                for b in range(NCTB):
                    nc.sync.dma_start(ctb[b][:, :, 1, :], G4_d[:])

                e_sb = {}
                rec = {}
                res_t = {}
                cont = constp.tile([BL, 2 * D], f32, tag="cont")
                sent = constp.tile([BL, 2 * D], f32, tag="sent")
                nc.sync.dma_start(sent[:, :D], senti_d[:])

                def _dots_and_einsum(k, nt, colbase, sgt0, gt0, gnt,
                                     tt, res, e_k, fgrps, fg_tiles):
                    # score dots: fp8 DoubleRow, 2 MMs per tile
                    # (chunk-pairs (0,1) and (2,3) accumulate in PSUM)
                    for j in range(gnt):
                        t = sgt0 + gt0 + j
                        col = colbase + t
                        for q in range(2):
                            nc.tensor.matmul(
                                scol[:, col : col + 1],
                                tt[:, 2 * q : 2 * q + 2, j * P : (j + 1) * P],
                                awT[k][:, :, q : q + 1],
                                start=(q == 0), stop=(q == 1),
                                perf_mode=DR,
                            )
                    # exp/lt/einsum for batches completed by this group
                    done_t = sgt0 + gt0 + gnt
                    for (bt0, bnb) in fgrps:
                        if bt0 + bnb <= done_t and bt0 + bnb > done_t - gnt:
                            nc.scalar.activation(
                                e_k[:, bt0 : bt0 + bnb],
                                scol[:, colbase + bt0 : colbase + bt0 + bnb],
                                AF.Exp,
                            )
                            fg, fnb = fg_tiles[bt0]
                            jj = bt0
                            while jj + 1 < bt0 + bnb:
                                lt2 = ltp.tile([P, 2, BL], f8, tag="lt2")
                                for i in range(2):
                                    nc.vector.tensor_scalar(
                                        lt2[:, i, :], I2f8[:],
                                        e_k[:, jj + i : jj + i + 1],
                                        None, OP.mult,
                                    )
                                nc.tensor.matmul(
                                    res[:BL, :],
                                    lt2[:],
                                    fg[:, jj - bt0 : jj - bt0 + 2, :],
                                    start=(jj == 0),
                                    stop=(jj + 2 >= nt),
                                    perf_mode=DR,
                                )
                                jj += 2
                            if jj < bt0 + bnb:  # odd single tail
                                lt1 = ltp.tile([P, 2, BL], f8, tag="lt2")
                                nc.vector.tensor_scalar(
                                    lt1[:, 0, :], I2f8[:],
                                    e_k[:, jj : jj + 1], None, OP.mult,
                                )
                                nc.tensor.matmul(
                                    res[:BL, :],
                                    lt1[:, 0, :],
                                    fg[:, jj - bt0, :],
                                    start=(jj == 0), stop=(jj + 1 >= nt),
                                )
                                jj += 1

                # ---------------- branches ----------------
                for k, nt, colbase in BRANCHES:
                    e_sb[k] = constp.tile([P, nt], f32, tag=f"e_{k}", name=f"e_{k}")
                    res = presp.tile([P, D], f32, tag="res")
                    res_t[k] = res
                    sgs = _tiles_split(nt, SG_TILES)
                    n_pairs = nt // 2
                    # einsum state
                    fgrps = _tiles_split(nt, EB_TILES)
                    fg_tiles = {}
                    for (ft0, fnb) in fgrps:
                        fg = fiop.tile([P, EB_TILES, D], f8, tag="f")
                        # SWDGE ring: keeps the f stream's FIFO independent of
                        # the pT stream (sync ring) so buffer-reuse waits on
                        # one ring can't block the other's head-of-line.
                        nc.gpsimd.dma_start(
                            fg[:, :fnb, :],
                            fR[k][ft0 * P : (ft0 + fnb) * P, :].rearrange(
                                "(a p) d -> p a d", p=P
                            ),
                        )
                        fg_tiles[ft0] = (fg, fnb)

                    ttg = {}
                    for sgi, (sgt0, sgnt) in enumerate(sgs):
                        buf = sgi % NCTB
                        nc.sync.dma_start(
                            ctb[buf][:, :, 0, : sgnt * P],
                            pT[k][:, :, sgt0 * P : (sgt0 + sgnt) * P],
                        )
                        grps = _tiles_split(sgnt, GRP_TILES)
                        # chunk-major over PAIRS of groups: back-to-back
                        # matmuls share the stationary A8 chunk (ldweights
                        # dedup), px bufs=2 holds both groups
                        gidx = 0
                        while gidx < len(grps):
                            pair = grps[gidx : gidx + 2]
                            gidx += len(pair)
                            pxs = [
                                pxp.tile([P, 4, GRP_TILES * P], f32, tag="px",
                                         name=f"px{gi}")
                                for gi in range(len(pair))
                            ]
                            for c in range(4):
                                for (px, (gt0, gnt)) in zip(pxs, pair):
                                    nc.tensor.matmul(
                                        px[:, c, : gnt * P],
                                        A8[k][:, :, c, :],
                                        ctb[buf][:, c, :, gt0 * P : (gt0 + gnt) * P],
                                        start=True, stop=True,
                                        perf_mode=DR,
                                    )
                            tts = []
                            for (px, (gt0, gnt)) in zip(pxs, pair):
                                R = gnt * P
                                tt = ttp.tile([P, 4, GRP_TILES * P], f8, tag="tt")
                                nc.scalar.activation(
                                    tt[:, :, :R], px[:, :, :R], AF.Tanh
                                )
                                tts.append(tt)
                            for ((gt0, gnt), tt) in zip(pair, tts):
                                _dots_and_einsum(k, nt, colbase, sgt0, gt0, gnt,
                                                 tt, res, e_sb[k], fgrps, fg_tiles)

                    # branch softmax denominator: per-batch sums via free-dim
                    # reduce + I2 partition-fold matmul
                    esum = smallp.tile([P, 1], f32, tag="esum")
                    nc.vector.tensor_reduce(esum[:], e_sb[k][:], axis=AX.X, op=OP.add)
                    esb = smallp.tile([P, 1], bf16, tag="esb")
                    nc.vector.tensor_copy(esb[:], esum[:])
                    fcol = 160 + 2 * len(rec)
                    nc.tensor.matmul(
                        scol[:BL, fcol : fcol + 1], I2bf[:], esb[:],
                        start=True, stop=True,
                    )
                    rc = constp.tile([BL, 1], f32, tag=f"rec_{k}")
                    nc.vector.reciprocal(rc[:], scol[:BL, fcol : fcol + 1])
                    rec[k] = rc
                    # drain res (scaled) so the res bank can rotate to the
                    # next branch
                    dst = {"c": cont[:, D:], "s": sent[:, D:], "a": cont[:, :D]}[k]
                    nc.vector.tensor_scalar(
                        dst, res[:BL, :], rc[:], None, OP.mult
                    )

                if mode != "full":
                    fin0 = constp.tile([BL, 2 * D], f32, tag="fin")
                    nc.vector.memset(fin0[:], 0.0)
                    nc.sync.dma_start(out_d[:], fin0[:])
                    continue

                # ---------------- gate ----------------
                wgc = wiop.tile([P, 8, D], bf16, tag="wgate")
                nc.scalar.dma_start(wgc[:], w_tc.rearrange("(c p) d -> p c d", p=P))
                wgs = wiop.tile([P, 8, D], bf16, tag="wgate")
                nc.scalar.dma_start(wgs[:], w_ts.rearrange("(c p) d -> p c d", p=P))
                wgh = wiop.tile([P, 4, D], bf16, tag="wproj")
                nc.scalar.dma_start(wgh[:], w_th.rearrange("(c p) d -> p c d", p=P))
                b3 = smallp.tile([3, D], bf16, tag="b3")
                nc.scalar.dma_start(b3[:], tb3_d[:])

                g_ps = presp.tile([P, D], f32, tag="res")
                first = True
                ti = 0
                for (src_sb, wt, cs) in (
                    (None, wgh, range(0, 4)),
                    (cont, wgc, range(0, 8)),
                    (sent, wgs, range(0, 8)),
                ):
                    for c in cs:
                        if src_sb is None:
                            lhsT_c = hT[:, c, :]
                        else:
                            tc0 = 256 + 64 * (ti % 2)
                            ti += 1
                            nc.tensor.transpose(
                                scol[:, tc0 : tc0 + 64],
                                src_sb[:, c * P : (c + 1) * P],
                                identf[:BL, :BL],
                            )
                            gt = ltp.tile([P, BL], bf16, tag="gT")
                            nc.vector.tensor_copy(gt[:], scol[:, tc0 : tc0 + 64])
                            lhsT_c = gt[:]
                        nc.tensor.matmul(
                            g_ps[:BL, :], lhsT_c, wt[:, c, :], start=first, stop=False
                        )
                        first = False
                nc.tensor.matmul(
                    g_ps[:BL, :], ones3[:], b3[:], start=False, stop=True
                )

                g_sb = smallp.tile([BL, D], bf16, tag="g_sb")
                nc.scalar.activation(g_sb[:], g_ps[:BL, :], AF.Tanh)
                gprod = smallp.tile([BL, D], bf16, tag="gprod")
                gacc = smallp.tile([BL, 1], f32, tag="gacc")
                nc.vector.scalar_tensor_tensor(
                    gprod[:], g_sb[:], 1.0, alphab[:], OP.mult, OP.mult,
                    accum_out=gacc[:],
                )
                gate = smallp.tile([BL, 1], f32, tag="gate")
                nc.scalar.activation(gate[:], gacc[:], AF.Sigmoid, bias=ab_col[:])

                diff = constp.tile([BL, 2 * D], f32, tag="diff")
                nc.vector.tensor_sub(diff[:], cont[:], sent[:])
                fin = constp.tile([BL, 2 * D], f32, tag="fin")
                nc.vector.scalar_tensor_tensor(
                    fin[:], diff[:], gate[:, 0:1], sent[:], OP.mult, OP.add
                )
                nc.sync.dma_start(out_d[:], fin[:])

    return nc


def _fixup_multiwait(nc):
    """Walrus allows only ONE sync wait per instruction (except
    InstEventSemaphore). Split extras onto same-engine NOPs."""
    from concourse import mybir

    nfix = 0
    for fn in nc.m.functions:
        for blk in fn.blocks:
            new = []
            for inst in blk.instructions:
                si = inst.sync_info
                waits = list(si.on_wait) if si is not None else []
                if len(waits) > 1 and type(inst).__name__ != "InstEventSemaphore":
                    for w in waits[:-1]:
                        nop = mybir.InstNoOp(
                            name=nc.get_next_instruction_name(), ins=[], outs=[]
                        )
                        nop.engine = inst.engine
                        nop.sync_info = mybir.SyncInfo(on_wait=[w], on_update=[])
                        nc.register_instruction(nop)
                        new.append(nop)
                        nfix += 1
                    si.on_wait = waits[-1:]
                new.append(inst)
            blk.instructions[:] = new
    return nfix


def _get_nc(reps=1, mode="full"):
    key = f"nc{reps}_{mode}"
    if key not in _CACHE:
        import concourse.bass as bass

        nc = bass.Bass()
        _build(nc, reps=reps, mode=mode)
        nc.finalize()
        _fixup_multiwait(nc)
        _CACHE[key] = nc
    return _CACHE[key]


def _make_in_maps(inputs):
    import ml_dtypes

    bf = ml_dtypes.bfloat16
    f8 = ml_dtypes.float8_e4m3
    f = lambda x: np.ascontiguousarray(np.asarray(x), dtype=np.float32)
    fb = lambda x: np.ascontiguousarray(np.asarray(x, dtype=np.float32).astype(bf))
    f8c = lambda x: np.ascontiguousarray(np.asarray(x, dtype=np.float32).astype(f8))

    def pT_prep(x):
        # [64, N, 512] -> [128, 4, N*64] fp8, r = n*64 + b, chunk-major d
        xT = np.asarray(x, np.float32).transpose(2, 1, 0)  # [512, N, 64]
        xT = xT.reshape(4, P, -1).transpose(1, 0, 2)  # [128, 4, N*64]
        return np.ascontiguousarray(xT.astype(f8))

    def fR_prep(x):
        # [64, N, 512] -> [(n b), 512] fp8
        return np.ascontiguousarray(
            np.asarray(x, np.float32).transpose(1, 0, 2).reshape(-1, D).astype(f8)
        )

    def awT_prep(aw):
        # [p, i, q] = aw[(2q+i)*128 + p], fp8 (DoubleRow ktile pairs)
        a4 = np.asarray(aw, np.float32).reshape(2, 2, P)  # [q, i, p]
        return np.ascontiguousarray(a4.transpose(2, 1, 0).astype(f8))

    I2 = np.zeros((P, BL), np.float32)
    for p in range(P):
        I2[p, p % BL] = 1.0
    G = np.zeros((P, SG_TILES * P), np.float32)
    r = np.arange(SG_TILES * P)
    G[r % BL, r] = 1.0
    G4 = np.broadcast_to(G[:, None, :], (P, 4, SG_TILES * P))
    A8base = np.zeros((P, 2, 4, P), np.float32)
    for c in range(4):
        A8base[:, 0, c, :] = np.eye(P)

    consts = {
        "identf32": np.eye(P, dtype=np.float32),
        "A8base": np.ascontiguousarray(A8base.astype(f8)),
        "I2f8": I2.astype(f8),
        "I2bf": I2.astype(bf),
        "G4": np.ascontiguousarray(G4.astype(f8)),
        "ones3": np.ones((3, BL), np.float32).astype(bf),
    }
    weights = {
        "c_h2cpt_w": fb(inputs["c_h2cpt_w"]),
        "c_h2cpt_b": fb(inputs["c_h2cpt_b"]).reshape(1, D),
        "s_h2word_w": fb(inputs["s_h2word_w"]),
        "s_h2word_b": fb(inputs["s_h2word_b"]).reshape(1, D),
        "c_h2att_w": fb(inputs["c_h2att_w"]),
        "c_h2att_b": fb(inputs["c_h2att_b"]).reshape(1, D),
        "awT_cpt": awT_prep(inputs["c_cptA_w"]),
        "awT_sw": awT_prep(inputs["s_wordA_w"]),
        "awT_att": awT_prep(inputs["c_attA_w"]),
        "t_cont_w": fb(inputs["t_cont_w"]),
        "t_senti_w": fb(inputs["t_senti_w"]),
        "t_h2att_w": fb(inputs["t_h2att_w"]),
        "tb3": np.stack(
            [
                fb(inputs["t_cont_b"]).reshape(D),
                fb(inputs["t_senti_b"]).reshape(D),
                fb(inputs["t_h2att_b"]).reshape(D),
            ]
        ),
        "t_alpha_w": fb(inputs["t_alpha_w"]).reshape(1, D),
        "t_alpha_b": fb(inputs["t_alpha_b"]).reshape(1, 1),
    }
    in_maps = []
    for i in range(M):
        sl = slice(i * BL, (i + 1) * BL)
        m = {
            "h": f(inputs["h"][sl]),
            "senti_feats": f(inputs["senti_feats"][sl]),
            "pT_cpt": pT_prep(inputs["p_cpt_feats"][sl]),
            "pT_sw": pT_prep(inputs["p_senti_word_feats"][sl]),
            "pT_att": pT_prep(inputs["p_att_feats"][sl]),
            "f_cpt": fR_prep(inputs["cpt_feats"][sl]),
            "f_sw": fR_prep(inputs["senti_word_feats"][sl]),
            "f_att": fR_prep(inputs["att_feats"][sl]),
        }
        m.update(weights)
        m.update(consts)
        in_maps.append(m)
    return in_maps


def _run(inputs, trace=False):
    from concourse.bass_utils import run_bass_kernel_spmd

    nc = _get_nc()
    in_maps = _make_in_maps(inputs)
    r = run_bass_kernel_spmd(nc, in_maps, core_ids=list(range(M)), trace=trace)
    out = np.concatenate([r.results[i]["out"] for i in range(M)], axis=0)
    return out, r


def kernel(**inputs):
    out, _ = _run(inputs, trace=False)
    return out
